# revision 37
# baseline (speedup 1.0000x reference)
"""AttentionDecoder (4-layer GPT block stack) on 8 trn2 NeuronCores.

Sharding: data-parallel over batch (4) x 2-way Megatron tensor-parallel.
Core pair (2b, 2b+1) owns batch element b; within a pair, core t takes
heads 8t..8t+7 (QKV column split), the matching w_proj rows, w_fc column
half and w_fcp row half, and w_f column half. Two bf16 AllReduces per
layer per pair (after attention c_proj and after MLP c_proj), pipelined
over four 256-row query chunks so collectives hide behind neighboring
chunks' compute.

On-core layout: activations are feature-major ("transposed", [D, S]) so
every matmul is out.T = W.T @ h.T with the weight k-tile stationary and
activations moving (N=256). LayerNorm gamma/beta are folded into the
downstream weights host-side; LN statistics (feature-dim sums) come from
ones-vector matmuls on the TensorEngine, with rows broadcast back across
partitions via K=1 ones-matmuls into PSUM. Softmax runs on transposed
scores (keys on partitions): no max-subtraction (scores are O(1) by
construction), additive -30000 mask tiles on diagonal blocks, exp on
ScalarE fused with the 1/sqrt(hd) scale, denominators via ones-matmul,
reciprocals via the fast DVE approximation.

All matmul operands are bf16 (fp32 PSUM accumulation); the residual
stream, LN/softmax statistics and the final output stay fp32.
"""

import sys

for _p in ("/opt/trn_rl_repo", "/root/.axon_site/_ro/trn_rl_repo"):
    if _p not in sys.path:
        sys.path.append(_p)

from contextlib import ExitStack

import numpy as np
import ml_dtypes

import concourse.bass as bass
import concourse.mybir as mybir
import concourse.tile as tile
from concourse import bacc
from concourse import bass_utils

F32 = mybir.dt.float32
BF16 = mybir.dt.bfloat16
NPBF16 = ml_dtypes.bfloat16

B, S, D, L = 4, 1024, 1024, 4
H, HD = 16, 64          # global heads, head dim
NH = H // 2             # heads per core (8)
P = 128                 # partitions
KT = D // P             # 8 k-tiles over the model dim
CH = 256                # query-chunk width (matmul moving dim)
NCH = S // CH           # 4 chunks
EPS = 1e-5
NEG = -30000.0
REPLICA_GROUPS = [[0, 1], [2, 3], [4, 5], [6, 7]]

AF = mybir.ActivationFunctionType
ADD = mybir.AluOpType.add


def _build_nc():
    nc = bacc.Bacc("TRN2", target_bir_lowering=False, debug=False,
                   num_devices=8)

    def inp(name, shape, dt=BF16):
        return nc.declare_dram_parameter(name, list(shape), dt, isOutput=False)

    xT_d = inp("xT", [D, S], BF16)
    wqk_d = [inp(f"wqk_{i}", [D, 2 * NH * HD]) for i in range(L)]
    wv_d = [inp(f"wv_{i}", [D, NH * HD]) for i in range(L)]
    wproj_d = [inp(f"wproj_{i}", [NH * HD, D]) for i in range(L)]
    wfc_d = [inp(f"wfc_{i}", [D, 2048]) for i in range(L)]
    wfcp_d = [inp(f"wfcp_{i}", [2048, D]) for i in range(L)]
    wf_d = inp("wf", [D, 512])
    bqk_d = [inp(f"bqk_{i}", [P, 8], F32) for i in range(L)]
    bv_d = [inp(f"bv_{i}", [1, 512], BF16) for i in range(L)]
    bproj_d = [inp(f"bproj_{i}", [P, 8], F32) for i in range(L)]
    bfc_d = [inp(f"bfc_{i}", [P, 16], F32) for i in range(L)]
    bfcp_d = [inp(f"bfcp_{i}", [P, 8], F32) for i in range(L)]
    bf_d = inp("bf", [P, 4], F32)
    mask_d = inp("masks", [2, P, CH], BF16)  # rel = 0, 128

    out_d = nc.declare_dram_parameter("out", [512, S], F32, isOutput=True)

    with tile.TileContext(nc) as tc, ExitStack() as ctx:
        resid = ctx.enter_context(tc.tile_pool(name="resid", bufs=1))
        wpool = ctx.enter_context(tc.tile_pool(name="wpool", bufs=1))
        spool = ctx.enter_context(tc.tile_pool(name="spool", bufs=1))
        hpool = ctx.enter_context(tc.tile_pool(name="hpool", bufs=28))
        qpool = ctx.enter_context(tc.tile_pool(name="qpool", bufs=16))
        kvpool = ctx.enter_context(tc.tile_pool(name="kvpool", bufs=1))
        epool = ctx.enter_context(tc.tile_pool(name="epool", bufs=12))
        apool = ctx.enter_context(tc.tile_pool(name="apool", bufs=10))
        mpool = ctx.enter_context(tc.tile_pool(name="mpool", bufs=16))
        sqpool = ctx.enter_context(tc.tile_pool(name="sqpool", bufs=4))
        rowpool = ctx.enter_context(tc.tile_pool(name="rowpool", bufs=2))
        bbpool = ctx.enter_context(tc.tile_pool(name="bbpool", bufs=2))
        arpool = ctx.enter_context(tc.tile_pool(name="arpool", bufs=8))
        opool = ctx.enter_context(tc.tile_pool(name="opool", bufs=1))
        ps_big = ctx.enter_context(tc.tile_pool(name="ps_big", bufs=5, space="PSUM"))
        ps_av = ctx.enter_context(tc.tile_pool(name="ps_av", bufs=1, space="PSUM"))
        dpool = ctx.enter_context(tc.tile_pool(name="dpool", bufs=6, space="DRAM"))

        # ---- constants ----
        ones_b = spool.tile([P, 1], BF16, tag="ones_b")
        nc.vector.memset(ones_b, 1.0)
        ones_row = spool.tile([1, P], BF16, tag="ones_row")
        nc.vector.memset(ones_row, 1.0)
        eps_t = spool.tile([1, 1], F32, tag="eps")
        nc.vector.memset(eps_t, EPS)
        mask_sb = [spool.tile([P, CH], BF16, tag=f"mask{j}", name=f"mask{j}")
                   for j in range(2)]
        for j in range(2):
            nc.sync.dma_start(out=mask_sb[j], in_=mask_d[j])

        # warm up the collective firmware while the prologue computes
        warm_sb = spool.tile([P, 4], BF16, tag="warm")
        nc.vector.memset(warm_sb, 0.0)
        warm_in = dpool.tile([P, 4], BF16, tag="warm_in", name="warm_in")
        warm_out = dpool.tile([P, 4], BF16, tag="warm_out", name="warm_out")
        nc.sync.dma_start(out=warm_in, in_=warm_sb)
        nc.gpsimd.collective_compute(
            "AllReduce", ADD, ins=[warm_in.opt()], outs=[warm_out.opt()],
            replica_groups=REPLICA_GROUPS)

        # ---- resident residual stream xT (fp32, feature-major) ----
        xT = [resid.tile([P, S], BF16, tag=f"xT{d}", name=f"xT{d}")
              for d in range(KT)]
        for d in range(KT):
            nc.gpsimd.dma_start(out=xT[d], in_=xT_d[d * P:(d + 1) * P, :])

        def ln_stats(c, red=None):
            """Fold residual (optional) + feature-dim sums into one PSUM row:
            cols 0:CH = sum(x), CH:2CH = sum(x^2)."""
            cs = bass.ds(c * CH, CH)
            if red is not None:
                for d in range(KT):
                    nc.vector.tensor_add(xT[d][:, cs], xT[d][:, cs], red[d])
            sq_tiles = []
            for d in range(KT):
                sq = sqpool.tile([P, CH], BF16, tag="sq", name="sq")
                nc.vector.tensor_mul(sq, xT[d][:, cs], xT[d][:, cs])
                sq_tiles.append(sq)
            st = ps_big.tile([1, 2 * CH], F32, tag="ps_big", name="st")
            for d in range(KT):
                nc.tensor.matmul(st[:, 0:CH], ones_b, xT[d][:, cs],
                                 start=(d == 0), stop=(d == KT - 1))
            for d in range(KT):
                nc.tensor.matmul(st[:, CH:2 * CH], ones_b, sq_tiles[d],
                                 start=(d == 0), stop=(d == KT - 1))
            return st

        def ln_finish(c, st, tag):
            """Row math + partition-broadcast + apply -> bf16 h tiles."""
            cs = bass.ds(c * CH, CH)
            mean = rowpool.tile([1, CH], F32, tag="rowA", name="mean")
            vb = rowpool.tile([1, CH], F32, tag="rowB", name="vb")
            vc = rowpool.tile([1, CH], F32, tag="rowC", name="vc")
            nc.vector.tensor_scalar_mul(mean, st[:, 0:CH], 1.0 / D)
            nc.vector.tensor_scalar_mul(vb, st[:, CH:2 * CH], 1.0 / D)
            nc.vector.tensor_mul(vc, mean, mean)
            nc.vector.tensor_sub(vb, vb, vc)              # var
            nc.scalar.activation(out=vc, in_=vb, func=AF.Sqrt, bias=eps_t)
            nc.vector.reciprocal_approx_fast(vb, vc)      # rstd
            nc.vector.tensor_mul(vc, mean, vb)
            nc.vector.tensor_scalar_mul(vc, vc, -1.0)     # -mean*rstd
            rstd_b = rowpool.tile([1, CH], BF16, tag="rowD", name="rstd_b")
            nmr_b = rowpool.tile([1, CH], BF16, tag="rowE", name="nmr_b")
            nc.vector.tensor_copy(rstd_b, vb)
            nc.vector.tensor_copy(nmr_b, vc)
            bc = ps_av.tile([P, 2 * CH], F32, tag="ps_bc", name="bc", bufs=2)
            nc.tensor.matmul(bc[:, 0:CH], ones_row, rstd_b, start=True, stop=True)
            nc.tensor.matmul(bc[:, CH:2 * CH], ones_row, nmr_b, start=True, stop=True)
            h_tiles = []
            for d in range(KT):
                t = sqpool.tile([P, CH], F32, tag="lnt", name="lnt", bufs=2)
                nc.vector.tensor_mul(t, xT[d][:, cs], bc[:, 0:CH])
                h = hpool.tile([P, CH], BF16, tag="h", name=f"h{tag}")
                nc.vector.tensor_add(h, t, bc[:, CH:2 * CH])
                h_tiles.append(h)
            return h_tiles

        def layernorm(c, tag, red=None):
            return ln_finish(c, ln_stats(c, red), tag)

        def all_reduce(src_sb):
            ar_in = dpool.tile([KT * P, CH], BF16, tag="ar_in", name="ar_in")
            ar_out = dpool.tile([KT * P, CH], BF16, tag="ar_out", name="ar_out")
            for m in range(KT):
                nc.sync.dma_start(out=ar_in[m * P:(m + 1) * P, :], in_=src_sb[m])
            nc.gpsimd.collective_compute(
                "AllReduce", ADD, ins=[ar_in.opt()], outs=[ar_out.opt()],
                replica_groups=REPLICA_GROUPS)
            red = []
            for m in range(KT):
                t = arpool.tile([P, CH], BF16, tag="ar_sb", name="ar_sb")
                nc.sync.dma_start(out=t, in_=ar_out[m * P:(m + 1) * P, :])
                red.append(t)
            return red

        # =================== layers ===================
        # pending[c]: AR_mlp result tiles not yet folded into xT[:, chunk c]
        pending = [None] * NCH


        for i in range(L):
            bqk_sb = spool.tile([P, 8], F32, tag="bqk", name="bqk")
            bproj_sb = spool.tile([P, 8], F32, tag="bproj", name="bproj")
            bfc_sb = spool.tile([P, 16], F32, tag="bfc", name="bfc")
            bfcp_sb = spool.tile([P, 8], F32, tag="bfcp", name="bfcp")
            bv_row = spool.tile([1, 512], BF16, tag="bv_row", name="bv_row")
            for sb, dr in ((bqk_sb, bqk_d[i]), (bproj_sb, bproj_d[i]),
                           (bfc_sb, bfc_d[i]), (bfcp_sb, bfcp_d[i]),
                           (bv_row, bv_d[i])):
                nc.sync.dma_start(out=sb, in_=dr.ap())
            bvB = bbpool.tile([P, 512], F32, tag="bvB", bufs=1, name="bvB")
            pbv = ps_av.tile([P, 512], F32, tag="ps_bc", name="pbv", bufs=2)
            nc.tensor.matmul(pbv, ones_row, bv_row, start=True, stop=True)
            nc.vector.tensor_copy(bvB, pbv)

            # layer-resident weights
            wqk_sb = [wpool.tile([P, 1024], BF16, tag=f"wqk{k}",
                                 name=f"wqk{k}_{i}") for k in range(KT)]
            wv_sb = [wpool.tile([P, 512], BF16, tag=f"wv{k}",
                                name=f"wv{k}_{i}") for k in range(KT)]
            wproj_sb = [wpool.tile([P, 1024], BF16, tag=f"wpj{k}",
                                   name=f"wpj{k}_{i}") for k in range(4)]
            wfc_sb = [wpool.tile([P, 2048], BF16, tag=f"wfc{k}",
                                 name=f"wfc{k}_{i}") for k in range(KT)]
            wfcp_sb = [wpool.tile([P, 1024], BF16, tag=f"wfp{k}",
                                  name=f"wfp{k}_{i}") for k in range(16)]
            for k in range(KT):
                nc.sync.dma_start(out=wqk_sb[k], in_=wqk_d[i][k * P:(k + 1) * P, :])
                nc.sync.dma_start(out=wv_sb[k], in_=wv_d[i][k * P:(k + 1) * P, :])
                nc.sync.dma_start(out=wfc_sb[k], in_=wfc_d[i][k * P:(k + 1) * P, :])
            for k in range(4):
                nc.sync.dma_start(out=wproj_sb[k], in_=wproj_d[i][k * P:(k + 1) * P, :])
            for k in range(16):
                nc.sync.dma_start(out=wfcp_sb[k], in_=wfcp_d[i][k * P:(k + 1) * P, :])

            kT_sb = [kvpool.tile([P, S], BF16, tag=f"kT{m}", name=f"kT{m}_{i}")
                     for m in range(4)]
            v_sb = [kvpool.tile([P, 512], BF16, tag=f"v{r}", name=f"v{r}_{i}")
                    for r in range(KT)]

            # ---- phase A: LN1 + QKV + V, stats one chunk ahead ----
            qT_c = {}
            stats_q = {}
            stats_q[0] = ln_stats(0, pending[0])
            pending[0] = None
            for c in range(NCH):
                cs = bass.ds(c * CH, CH)
                if c + 1 < NCH:
                    stats_q[c + 1] = ln_stats(c + 1, pending[c + 1])
                    pending[c + 1] = None
                h1 = ln_finish(c, stats_q.pop(c), tag="1")
                qT = []
                for m in range(KT):
                    pmm = ps_big.tile([P, CH], F32, tag="ps_big", name="pqkv")
                    for k in range(KT):
                        nc.tensor.matmul(pmm, wqk_sb[k][:, m * P:(m + 1) * P],
                                         h1[k], start=(k == 0), stop=(k == KT - 1))
                    if m < 4:
                        q = qpool.tile([P, CH], BF16, tag="qT", name="qT")
                        nc.vector.tensor_scalar(out=q, in0=pmm,
                                                scalar1=bqk_sb[:, m:m + 1],
                                                scalar2=None, op0=ADD)
                        qT.append(q)
                    else:
                        nc.vector.tensor_scalar(out=kT_sb[m - 4][:, cs], in0=pmm,
                                                scalar1=bqk_sb[:, m:m + 1],
                                                scalar2=None, op0=ADD)
                qT_c[c] = qT
                for r in range(CH // P):
                    pmm = ps_big.tile([P, 512], F32, tag="ps_big", name="pv")
                    for k in range(KT):
                        nc.tensor.matmul(pmm,
                                         h1[k][:, r * P:(r + 1) * P],
                                         wv_sb[k],
                                         start=(k == 0), stop=(k == KT - 1))
                    nc.vector.tensor_add(v_sb[c * (CH // P) + r], pmm, bvB)

            # ---- phase B: attention, with c_proj+AllReduce one chunk behind
            def attention(c):
                qT = qT_c[c]
                n_kt = (c + 1) * (CH // P)
                attnT = []
                for hp in range(4):
                    at = apool.tile([P, CH], BF16, tag="attnT", name="attnT")
                    exps_h = {}
                    psum_h = {}
                    recip_h = {}
                    for h in range(2):
                        hs = bass.ds(h * HD, HD)
                        exps = []
                        for kt in range(n_kt):
                            pss = ps_big.tile([P, CH], F32, tag="ps_big",
                                              name="pss")
                            nc.tensor.matmul(pss,
                                             kT_sb[hp][hs, kt * P:(kt + 1) * P],
                                             qT[hp][hs, :],
                                             start=True, stop=True,
                                             tile_position=(h * HD, 0))
                            rel = kt * P - c * CH
                            if rel >= 0:
                                nc.vector.tensor_add(pss, pss,
                                                     mask_sb[rel // P])
                            e = epool.tile([P, CH], BF16, tag="expp",
                                           name="expp")
                            nc.scalar.activation(out=e, in_=pss, func=AF.Exp,
                                                 scale=0.125)
                            exps.append(e)
                        exps_h[h] = exps
                    for h in range(2):
                        psum = ps_big.tile([1, CH], F32, tag="ps_big",
                                           name="psum_s")
                        for j, e in enumerate(exps_h[h]):
                            nc.tensor.matmul(psum, ones_b, e, start=(j == 0),
                                             stop=(j == len(exps_h[h]) - 1))
                        recip = rowpool.tile([1, CH], F32, tag="rowA",
                                             name="recip")
                        nc.vector.reciprocal_approx_fast(recip, psum)
                        recip_b = rowpool.tile([1, CH], BF16, tag="rowD",
                                               name="recip_b")
                        nc.vector.tensor_copy(recip_b, recip)
                        recip_h[h] = recip_b
                    for h in range(2):
                        lh = 2 * hp + h
                        hs = bass.ds(h * HD, HD)
                        pav = ps_av.tile([HD, CH], F32, tag="ps_av", name="pav")
                        for j, e in enumerate(exps_h[h]):
                            nc.tensor.matmul(pav,
                                             v_sb[j][:, lh * HD:(lh + 1) * HD],
                                             e, start=(j == 0),
                                             stop=(j == len(exps_h[h]) - 1))
                        prB = ps_av.tile([HD, CH], F32, tag="ps_bc",
                                         name="prB", bufs=2)
                        nc.tensor.matmul(prB, ones_row[:, :HD], recip_h[h],
                                         start=True, stop=True)
                        rB = bbpool.tile([HD, CH], F32, tag="rB", name="rB")
                        nc.vector.tensor_copy(rB, prB)
                        nc.vector.tensor_mul(at[hs, :], pav, rB)
                    attnT.append(at)
                return attnT

            attn_red = {}

            def proj_ar_attn(c, attnT):
                prj = []
                for m in range(KT):
                    pmm = ps_big.tile([P, CH], F32, tag="ps_big", name="pprj")
                    for k in range(4):
                        nc.tensor.matmul(pmm, wproj_sb[k][:, m * P:(m + 1) * P],
                                         attnT[k], start=(k == 0), stop=(k == 3))
                    t = arpool.tile([P, CH], BF16, tag="arin", name="prj")
                    nc.vector.tensor_scalar(out=t, in0=pmm,
                                            scalar1=bproj_sb[:, m:m + 1],
                                            scalar2=None, op0=ADD)
                    prj.append(t)
                attn_red[c] = all_reduce(prj)

            prev_attn = None
            for c in range(NCH):
                at = attention(c)
                if prev_attn is not None:
                    proj_ar_attn(prev_attn[0], prev_attn[1])
                prev_attn = (c, at)
            proj_ar_attn(prev_attn[0], prev_attn[1])

            # ---- phase C: MLP per chunk, LN2 stats one chunk ahead ----
            stats2 = {}
            stats2[0] = ln_stats(0, attn_red[0])
            for c in range(NCH):
                cs = bass.ds(c * CH, CH)
                if c + 1 < NCH:
                    stats2[c + 1] = ln_stats(c + 1, attn_red[c + 1])
                h2 = ln_finish(c, stats2.pop(c), tag="2")
                mt = []
                for m in range(16):
                    pmm = ps_big.tile([P, CH], F32, tag="ps_big", name="pfc")
                    for k in range(KT):
                        nc.tensor.matmul(pmm, wfc_sb[k][:, m * P:(m + 1) * P],
                                         h2[k], start=(k == 0), stop=(k == KT - 1))
                    t = mpool.tile([P, CH], BF16, tag="mt", name="mt")
                    nc.scalar.activation(out=t, in_=pmm, func=AF.Relu,
                                         bias=bfc_sb[:, m:m + 1], scale=1.0)
                    mt.append(t)
                fcp = []
                for m in range(KT):
                    pmm = ps_big.tile([P, CH], F32, tag="ps_big", name="pfcp")
                    for k in range(16):
                        nc.tensor.matmul(pmm, wfcp_sb[k][:, m * P:(m + 1) * P],
                                         mt[k], start=(k == 0), stop=(k == 15))
                    t = arpool.tile([P, CH], BF16, tag="arin", name="fcp")
                    nc.vector.tensor_scalar(out=t, in0=pmm,
                                            scalar1=bfcp_sb[:, m:m + 1],
                                            scalar2=None, op0=ADD)
                    fcp.append(t)
                pending[c] = all_reduce(fcp)

        # =================== final LN + head ===================
        bf_sb = spool.tile([P, 4], F32, tag="bf", name="bf_sb")
        nc.sync.dma_start(out=bf_sb, in_=bf_d.ap())
        wf_sb = [wpool.tile([P, 512], BF16, tag=f"wv{k}", name=f"wf{k}")
                 for k in range(KT)]
        for k in range(KT):
            nc.sync.dma_start(out=wf_sb[k], in_=wf_d[k * P:(k + 1) * P, :])
        stats_f = {}
        stats_f[0] = ln_stats(0, pending[0])
        pending[0] = None
        for c in range(NCH):
            cs = bass.ds(c * CH, CH)
            if c + 1 < NCH:
                stats_f[c + 1] = ln_stats(c + 1, pending[c + 1])
                pending[c + 1] = None
            hf = ln_finish(c, stats_f.pop(c), tag="f")
            for m in range(4):
                pmm = ps_big.tile([P, CH], F32, tag="ps_big", name="phead")
                for k in range(KT):
                    nc.tensor.matmul(pmm, wf_sb[k][:, m * P:(m + 1) * P],
                                     hf[k], start=(k == 0), stop=(k == KT - 1))
                t = opool.tile([P, CH], F32, tag="oh", name="oh")
                nc.vector.tensor_scalar(out=t, in0=pmm,
                                        scalar1=bf_sb[:, m:m + 1],
                                        scalar2=None, op0=ADD)
                nc.sync.dma_start(out=out_d[m * P:(m + 1) * P, cs], in_=t)

    nc.compile()
    return nc


_NC_CACHE = None


def _get_nc():
    global _NC_CACHE
    if _NC_CACHE is None:
        _NC_CACHE = _build_nc()
    return _NC_CACHE


def _rearr_vec(v):
    """[n*128] feature vector -> [128, n] (feature = m*128 + p)."""
    n = v.shape[0] // P
    return np.ascontiguousarray(v.reshape(n, P).T).astype(np.float32)


def _make_masks():
    m = np.zeros((2, P, CH), np.float32)
    for j in range(2):
        rel = j * P
        p = np.arange(P)[:, None]
        f = np.arange(CH)[None, :]
        m[j] = np.where(rel + p <= f, 0.0, NEG)
    return m.astype(NPBF16)


def _shard_inputs(x, ln1_g, ln1_b, w_attn, b_attn, w_proj, b_proj,
                  ln2_g, ln2_b, w_fc, b_fc, w_fcp, b_fcp,
                  lnf_g, lnf_b, w_f, b_f):
    bf = lambda a: np.ascontiguousarray(a).astype(NPBF16)
    masks = _make_masks()
    in_maps = []
    for core in range(8):
        b, t = core // 2, core % 2
        hsl = slice(t * NH * HD, (t + 1) * NH * HD)
        m = {"xT": np.ascontiguousarray(x[b].T).astype(NPBF16),
             "masks": masks,
             "wf": bf((w_f * lnf_g[:, None])[:, t * 512:(t + 1) * 512]),
             "bf": _rearr_vec((b_f + lnf_b @ w_f)[t * 512:(t + 1) * 512])}
        for i in range(L):
            g1, b1 = ln1_g[i], ln1_b[i]
            g2, b2 = ln2_g[i], ln2_b[i]
            wa = w_attn[i] * g1[:, None]                   # fold LN1 gamma
            ba = b_attn[i] + b1 @ w_attn[i]                # fold LN1 beta
            wq, wk, wv = wa[:, :D], wa[:, D:2 * D], wa[:, 2 * D:]
            bq, bk, bv = ba[:D], ba[D:2 * D], ba[2 * D:]
            m[f"wqk_{i}"] = bf(np.concatenate([wq[:, hsl], wk[:, hsl]], axis=1))
            m[f"wv_{i}"] = bf(wv[:, hsl])
            m[f"bqk_{i}"] = _rearr_vec(np.concatenate([bq[hsl], bk[hsl]]))
            m[f"bv_{i}"] = bv[hsl].reshape(1, 512).astype(NPBF16)
            m[f"wproj_{i}"] = bf(w_proj[i][hsl, :])
            m[f"bproj_{i}"] = _rearr_vec(b_proj[i] * 0.5)  # split across pair
            wfc_f = w_fc[i] * g2[:, None]
            bfc_f = b_fc[i] + b2 @ w_fc[i]
            m[f"wfc_{i}"] = bf(wfc_f[:, t * 2048:(t + 1) * 2048])
            m[f"bfc_{i}"] = _rearr_vec(bfc_f[t * 2048:(t + 1) * 2048])
            m[f"wfcp_{i}"] = bf(w_fcp[i][t * 2048:(t + 1) * 2048, :])
            m[f"bfcp_{i}"] = _rearr_vec(b_fcp[i] * 0.5)
        in_maps.append(m)
    return in_maps


def kernel(**inputs):
    nc = _get_nc()
    in_maps = _shard_inputs(**inputs)
    res = bass_utils.run_bass_kernel_spmd(nc, in_maps, core_ids=list(range(8)))
    outs = res.results
    full = np.empty((B, S, D), np.float32)
    for core in range(8):
        b, t = core // 2, core % 2
        full[b][:, t * 512:(t + 1) * 512] = outs[core]["out"].T
    return full


if __name__ == "__main__":
    nc = _get_nc()
    print("built ok;",
          sum(len(bb.instructions) for bb in nc.main_func.blocks
              if hasattr(bb, "instructions")), "instructions")
